# revision 9
# baseline (speedup 1.0000x reference)
"""Gaussian KDE (brute-force, bandwidth^2 = 1) on 8 Trainium2 NeuronCores.

Math:
    out_i = log( sum_j w_j * exp(-||x_i - y_j||^2 / 2) ) - (d/2) log(2pi) - log(sum_j w_j)
          = log( sum_j exp(x_i . y_j + b_j - m_i) ) + m_i - ||x_i||^2/2 - consts
    with b_j = log(w_j) - ||y_j||^2/2 and m_i a per-query shift that keeps the
    exponentials inside fp8-e5m2 dynamic range.

Flipped-orientation design (trains on PSUM partitions, queries on free dim):
    - scores: per 128-train tile, one K=35 bf16 matmul
      (rows = 32 y dims + bias hi + bias lo + ones; moving x rows =
       32 x dims + 1 + 1 + (-m_i - AOFF)). K=35 <= 64 so two matmuls run
      concurrently on disjoint PE row groups via tile_position (0,0)/(64,0).
    - exp: ScalarE table-exp (4 tiles per ACTIVATE) and VectorE
      int8-Schraudolph (3 tiles per TENSOR_SCALAR; the int8 affine result
      IS the e5m2 bit pattern of exp) write fp8-e5m2 values to SBUF.
    - reduce over trains: TensorE ones-matmul in fp8 DoubleRow perf mode
      (2 train-tiles per matmul at 0.5 cycles/column), accumulating into a
      persistent PSUM bank across all 512 train tiles.
    - final: ln on ScalarE, subtract per-query constant, DMA out.
"""

import numpy as np
import ml_dtypes

_Q, _N, _D = 4096, 65536, 32
_NCORES = 8
_QSHARD = _Q // _NCORES          # 512 queries per core
_K = 35                          # 32 dims + bias hi + bias lo + shift row
_TT = _N // 128                  # 512 train tiles per core
_RT = 7                          # tiles per round: 4 ACT + 3 DVE
_NR = _TT // _RT                 # 73 full rounds, 1 leftover tile

_BF16 = ml_dtypes.bfloat16

# Per-query shift estimate m_i ~= A_FIT*||x_i|| + C_FIT (keeps exp args in
# e5m2 range; exactness is irrelevant - m is added back exactly at the end).
_A_FIT = 4.42465707
_C_FIT = -17.07362259
# exp arg = s - m_i - AOFF; AOFF places the largest arg ~1 nat under ln(57344).
_AOFF = 8.0 + 1.0 - float(np.log(57344.0))

# Schraudolph fast-exp producing e5m2 bits: e5m2(e^s) ~ uint8(C1*s + C2).
_C1 = 4.0 / float(np.log(2.0))


def _c2_mean_zero():
    f = (np.arange(100000, dtype=np.float64) + 0.5) / 100000.0
    m0 = np.mean((1.0 + f) * 2.0 ** (-f))
    m1 = np.mean(2.0 ** (-f))
    delta = (m0 - 1.0) / m1
    return float(15 * 4.0 - delta * 4.0)


_C2 = _c2_mean_zero()

_USE_DR = True
_ALL_ACT = True

_prog_cache: dict = {}


def _build_program(n_trains: int):
    import concourse.bass as bass
    import concourse.tile as tile
    from concourse import bacc, mybir

    f32 = mybir.dt.float32
    bf16 = mybir.dt.bfloat16
    i8 = mybir.dt.int8
    f8 = mybir.dt.float8e5
    DR = mybir.MatmulPerfMode.DoubleRow

    nc = bacc.Bacc("TRN2", target_bir_lowering=False, debug=False,
                   num_devices=_NCORES)

    y_d = nc.dram_tensor("yext", [_K, n_trains], bf16, kind="ExternalInput")
    on_d = nc.dram_tensor("ones8", [128, 256], mybir.dt.int8, kind="ExternalInput")
    x_d = nc.dram_tensor("xext", [_K, _QSHARD], bf16, kind="ExternalInput")
    dv_d = nc.dram_tensor("dv", [1, _QSHARD], f32, kind="ExternalInput")
    out_d = nc.dram_tensor("out", [1, _QSHARD], f32, kind="ExternalOutput")

    ntt = n_trains // 128
    nr = ntt // _RT
    nleft = ntt - nr * _RT

    with tile.TileContext(nc) as tc:
        with (
            tc.tile_pool(name="const", bufs=1) as cpool,
            tc.tile_pool(name="y", bufs=3) as ypool,
            tc.tile_pool(name="ea", bufs=2) as eapool,
            tc.tile_pool(name="ed", bufs=2) as edpool,
            tc.tile_pool(name="small", bufs=2) as spool,
            tc.tile_pool(name="ps", bufs=1, space="PSUM") as ppool,
            tc.tile_pool(name="acc", bufs=1, space="PSUM") as apool,
        ):
            xsb = cpool.tile([128, _QSHARD], bf16)
            nc.sync.dma_start(xsb[0:_K, :], x_d[:])
            nc.sync.dma_start(xsb[64:64 + _K, :], x_d[:])
            dv_sb = cpool.tile([1, _QSHARD], f32)
            nc.sync.dma_start(dv_sb[:], dv_d[:])
            ones_dr = cpool.tile([128, 2, 128], f8)
            nc.sync.dma_start(ones_dr[:].bitcast(i8), on_d[:])
            ones_pl = cpool.tile([128, 128], f8)
            nc.sync.dma_start(ones_pl[:].bitcast(i8), on_d[:, 0:128])

            # 7 score banks + 1 accumulator bank = all 8 PSUM banks
            ps = ppool.tile([128, _RT * 512], f32)
            acc = apool.tile([128, _QSHARD], f32)

            first_red = [True]

            def reduce_dr(rhs_ap, last=False):
                nc.tensor.matmul(out=acc[:], lhsT=ones_dr[:], rhs=rhs_ap,
                                 start=first_red[0], stop=last,
                                 perf_mode=DR, skip_group_check=True)
                first_red[0] = False

            def reduce_plain(rhs_ap, last=False):
                nc.tensor.matmul(out=acc[:], lhsT=ones_pl[:], rhs=rhs_ap,
                                 start=first_red[0], stop=last,
                                 skip_group_check=True)
                first_red[0] = False

            for r in range(nr):
                t0 = r * _RT
                ych = ypool.tile([128, _RT * 128], bf16)
                nc.sync.dma_start(ych[0:_K, :],
                                  y_d[:, t0 * 128:(t0 + _RT) * 128])
                nc.sync.dma_start(ych[64:64 + _K, :],
                                  y_d[:, t0 * 128:(t0 + _RT) * 128])
                for i in range(_RT):
                    rg = 64 * (i % 2)
                    nc.tensor.matmul(
                        out=ps[:, i * 512:(i + 1) * 512],
                        lhsT=ych[rg:rg + _K, i * 128:(i + 1) * 128],
                        rhs=xsb[rg:rg + _K, :],
                        start=True, stop=True,
                        tile_position=(rg, 0),
                    )
                ea = eapool.tile([128, 4, _QSHARD], f8)
                nc.scalar.activation(ea[:], ps[:, 0:2048],
                                     mybir.ActivationFunctionType.Exp)
                ed = edpool.tile([128, 3, _QSHARD], i8)
                nc.vector.tensor_scalar(
                    ed[:], ps[:, 2048:3584], _C1, _C2,
                    mybir.AluOpType.mult, mybir.AluOpType.add)
                if _ALL_ACT:
                    ea2 = eapool.tile([128, 3, _QSHARD], f8)
                    nc.scalar.activation(ea2[:], ps[:, 2048:3584],
                                         mybir.ActivationFunctionType.Exp)
                    reduce_dr(ea[:, 0:2, :])
                    reduce_dr(ea[:, 2:4, :])
                    reduce_dr(ea2[:, 0:2, :])
                    reduce_plain(ea2[:, 2, :],
                                 last=(r == nr - 1 and nleft == 0))
                elif _USE_DR:
                    reduce_dr(ea[:, 0:2, :])
                    reduce_dr(ea[:, 2:4, :])
                    reduce_dr(ed[:, 0:2, :].bitcast(f8))
                    reduce_plain(ed[:, 2, :].bitcast(f8),
                                 last=(r == nr - 1 and nleft == 0))
                else:
                    for p in range(4):
                        reduce_plain(ea[:, p, :])
                    for p in range(2):
                        reduce_plain(ed[:, p, :].bitcast(f8))
                    reduce_plain(ed[:, 2, :].bitcast(f8),
                                 last=(r == nr - 1 and nleft == 0))

            if nleft:
                t0 = nr * _RT
                ych = ypool.tile([128, nleft * 128], bf16)
                nc.sync.dma_start(ych[0:_K, :], y_d[:, t0 * 128:])
                for i in range(nleft):
                    nc.tensor.matmul(
                        out=ps[:, i * 512:(i + 1) * 512],
                        lhsT=ych[0:_K, i * 128:(i + 1) * 128],
                        rhs=xsb[0:_K, :],
                        start=True, stop=True,
                        tile_position=(0, 0),
                    )
                ea = eapool.tile([128, nleft, _QSHARD], f8)
                nc.scalar.activation(ea[:], ps[:, 0:nleft * 512],
                                     mybir.ActivationFunctionType.Exp)
                for i in range(nleft):
                    reduce_plain(ea[:, i, :], last=(i == nleft - 1))

            lg = spool.tile([1, _QSHARD], f32)
            nc.scalar.activation(lg[:], acc[0:1, :],
                                 mybir.ActivationFunctionType.Ln)
            fin = spool.tile([1, _QSHARD], f32)
            nc.vector.tensor_tensor(fin[:], lg[:], dv_sb[:],
                                    mybir.AluOpType.subtract)
            nc.sync.dma_start(out_d[:], fin[:])

    nc.compile()
    return nc


def _get_program(n_trains: int):
    if n_trains not in _prog_cache:
        _prog_cache[n_trains] = _build_program(n_trains)
    return _prog_cache[n_trains]


def _prep_inputs(X, X_train, sample_weight):
    X = np.ascontiguousarray(np.asarray(X, dtype=np.float32))
    Y = np.ascontiguousarray(np.asarray(X_train, dtype=np.float32))
    w = np.ascontiguousarray(np.asarray(sample_weight, dtype=np.float32))
    n = Y.shape[0]

    w64 = w.astype(np.float64)
    b64 = np.log(np.maximum(w64, 1e-300)) - 0.5 * np.sum(
        Y.astype(np.float64) ** 2, axis=1)
    b64 = np.clip(b64, -35.0, None)
    bhi = b64.astype(np.float32).astype(_BF16)
    blo = (b64 - bhi.astype(np.float64)).astype(np.float32).astype(_BF16)

    yext = np.empty((_K, n), dtype=_BF16)
    yext[0:32] = Y.astype(_BF16).T
    yext[32] = bhi
    yext[33] = blo
    yext[34] = np.ones(n, dtype=_BF16)

    const = 0.5 * _D * np.log(2.0 * np.pi) + np.log(np.sum(w64))
    xsq = np.sum(X.astype(np.float64) ** 2, axis=1)
    r = np.sqrt(xsq)
    m_est = _A_FIT * r + _C_FIT
    mrow = (-(m_est + _AOFF)).astype(np.float32).astype(_BF16)   # [Q]
    # out = ln(total) - (mrow + ||x||^2/2 + const)
    dv_all = (mrow.astype(np.float64) + 0.5 * xsq + const).astype(np.float32)

    in_maps = []
    for c in range(_NCORES):
        sl = slice(c * _QSHARD, (c + 1) * _QSHARD)
        xq = X[sl]
        xext = np.empty((_K, _QSHARD), dtype=_BF16)
        xext[0:32] = xq.astype(_BF16).T
        xext[32] = np.ones(_QSHARD, dtype=_BF16)
        xext[33] = np.ones(_QSHARD, dtype=_BF16)
        xext[34] = mrow[sl]
        dv = np.ascontiguousarray(dv_all[sl].reshape(1, _QSHARD))
        ones8 = np.full((128, 256), 0x3c, dtype=np.int8)  # e5m2 1.0
        in_maps.append({"yext": yext, "xext": xext, "dv": dv, "ones8": ones8})
    return in_maps


def _gather(results):
    out = np.empty(_Q, dtype=np.float32)
    for c in range(_NCORES):
        out[c * _QSHARD:(c + 1) * _QSHARD] = results[c]["out"].reshape(-1)
    return out


def kernel(X, X_train, sample_weight, _want_timing=False):
    from concourse.bass_utils import run_bass_kernel_spmd

    nc = _get_program(_N)
    in_maps = _prep_inputs(X, X_train, sample_weight)
    kres = run_bass_kernel_spmd(
        nc, in_maps, core_ids=list(range(_NCORES)),
        trace=bool(_want_timing),
    )
    out = _gather(kres.results)
    if _want_timing:
        return out, kres
    return out


# revision 11
# speedup vs baseline: 1.2355x; 1.2355x over previous
"""Gaussian KDE (brute-force, bandwidth^2 = 1) on 8 Trainium2 NeuronCores.

Math:
    out_i = log( sum_j w_j * exp(-||x_i - y_j||^2 / 2) ) - (d/2) log(2pi) - log(sum_j w_j)
          = log( sum_j exp(x_i . y_j + b_j - m_i) ) + m_i - ||x_i||^2/2 - consts
    with b_j = log(w_j) - ||y_j||^2/2 and m_i a per-query shift that keeps the
    exponentials inside fp8-e5m2 dynamic range.

Flipped-orientation design (trains on PSUM partitions, queries on free dim):
    - scores: per 128-train tile, one K=35 bf16 matmul
      (rows = 32 y dims + bias hi + bias lo + ones; moving x rows =
       32 x dims + 1 + 1 + (-m_i - AOFF)). K=35 <= 64 so two matmuls run
      concurrently on disjoint PE row groups via tile_position (0,0)/(64,0).
    - exp: ScalarE table-exp (4 tiles per ACTIVATE) and VectorE
      int8-Schraudolph (3 tiles per TENSOR_SCALAR; the int8 affine result
      IS the e5m2 bit pattern of exp) write fp8-e5m2 values to SBUF.
    - reduce over trains: TensorE ones-matmul in fp8 DoubleRow perf mode
      (2 train-tiles per matmul at 0.5 cycles/column), accumulating into a
      persistent PSUM bank across all 512 train tiles.
    - final: ln on ScalarE, subtract per-query constant, DMA out.
"""

import numpy as np
import ml_dtypes

_Q, _N, _D = 4096, 65536, 32
_NCORES = 8
_QSHARD = _Q // _NCORES          # 512 queries per core
_K = 35                          # 32 dims + bias hi + bias lo + shift row
_TT = _N // 128                  # 512 train tiles per core
_RT = 7                          # tiles per round: 4 ACT + 3 DVE
_NR = _TT // _RT                 # 73 full rounds, 1 leftover tile

_BF16 = ml_dtypes.bfloat16

# Per-query shift estimate m_i ~= A_FIT*||x_i|| + C_FIT (keeps exp args in
# e5m2 range; exactness is irrelevant - m is added back exactly at the end).
_A_FIT = 4.42465707
_C_FIT = -17.07362259
# exp arg = s - m_i - AOFF; AOFF places the largest arg ~1 nat under ln(57344).
_AOFF = 8.0 + 1.0 - float(np.log(57344.0))

# Schraudolph fast-exp producing e5m2 bits: e5m2(e^s) ~ uint8(C1*s + C2).
_C1 = 4.0 / float(np.log(2.0))


def _c2_mean_zero():
    f = (np.arange(100000, dtype=np.float64) + 0.5) / 100000.0
    m0 = np.mean((1.0 + f) * 2.0 ** (-f))
    m1 = np.mean(2.0 ** (-f))
    delta = (m0 - 1.0) / m1
    return float(15 * 4.0 - delta * 4.0)


_C2 = _c2_mean_zero()

_USE_DR = True
_ALL_ACT = False

_prog_cache: dict = {}


def _build_program(n_trains: int):
    import concourse.bass as bass
    import concourse.tile as tile
    from concourse import bacc, mybir

    f32 = mybir.dt.float32
    bf16 = mybir.dt.bfloat16
    i8 = mybir.dt.int8
    f8 = mybir.dt.float8e5
    DR = mybir.MatmulPerfMode.DoubleRow

    nc = bacc.Bacc("TRN2", target_bir_lowering=False, debug=False,
                   num_devices=_NCORES)

    y_d = nc.dram_tensor("yext", [_K, n_trains], bf16, kind="ExternalInput")
    on_d = nc.dram_tensor("ones8", [128, 256], mybir.dt.int8, kind="ExternalInput")
    x_d = nc.dram_tensor("xext", [_K, _QSHARD], bf16, kind="ExternalInput")
    dv_d = nc.dram_tensor("dv", [1, _QSHARD], f32, kind="ExternalInput")
    out_d = nc.dram_tensor("out", [1, _QSHARD], f32, kind="ExternalOutput")

    ntt = n_trains // 128
    nr = ntt // _RT
    nleft = ntt - nr * _RT

    with tile.TileContext(nc) as tc:
        with (
            tc.tile_pool(name="const", bufs=1) as cpool,
            tc.tile_pool(name="y", bufs=3) as ypool,
            tc.tile_pool(name="ea", bufs=2) as eapool,
            tc.tile_pool(name="ed", bufs=2) as edpool,
            tc.tile_pool(name="small", bufs=2) as spool,
            tc.tile_pool(name="ps", bufs=1, space="PSUM") as ppool,
            tc.tile_pool(name="acc", bufs=1, space="PSUM") as apool,
        ):
            xsb = cpool.tile([128, _QSHARD], bf16)
            nc.sync.dma_start(xsb[0:_K, :], x_d[:])
            nc.sync.dma_start(xsb[64:64 + _K, :], x_d[:])
            dv_sb = cpool.tile([1, _QSHARD], f32)
            nc.sync.dma_start(dv_sb[:], dv_d[:])
            ones_dr = cpool.tile([128, 2, 128], f8)
            nc.sync.dma_start(ones_dr[:].bitcast(i8), on_d[:])
            ones_pl = cpool.tile([128, 128], f8)
            nc.sync.dma_start(ones_pl[:].bitcast(i8), on_d[:, 0:128])

            # 7 score banks + 1 accumulator bank = all 8 PSUM banks
            ps = ppool.tile([128, _RT * 512], f32)
            acc = apool.tile([128, _QSHARD], f32)

            first_red = [True]

            def reduce_dr(rhs_ap, last=False):
                nc.tensor.matmul(out=acc[:], lhsT=ones_dr[:], rhs=rhs_ap,
                                 start=first_red[0], stop=last,
                                 perf_mode=DR, skip_group_check=True)
                first_red[0] = False

            def reduce_plain(rhs_ap, last=False):
                nc.tensor.matmul(out=acc[:], lhsT=ones_pl[:], rhs=rhs_ap,
                                 start=first_red[0], stop=last,
                                 skip_group_check=True)
                first_red[0] = False

            for r in range(nr):
                t0 = r * _RT
                ych = ypool.tile([128, _RT * 128], bf16)
                nc.sync.dma_start(ych[0:_K, :],
                                  y_d[:, t0 * 128:(t0 + _RT) * 128])
                nc.sync.dma_start(ych[64:64 + _K, :],
                                  y_d[:, t0 * 128:(t0 + _RT) * 128])
                for i in range(_RT):
                    rg = 64 * (i % 2)
                    nc.tensor.matmul(
                        out=ps[:, i * 512:(i + 1) * 512],
                        lhsT=ych[rg:rg + _K, i * 128:(i + 1) * 128],
                        rhs=xsb[rg:rg + _K, :],
                        start=True, stop=True,
                        tile_position=(rg, 0),
                    )
                ea = eapool.tile([128, 4, _QSHARD], f8)
                nc.scalar.activation(ea[:], ps[:, 0:2048],
                                     mybir.ActivationFunctionType.Exp,
                                     scale=1.0 / _C1)
                ed = edpool.tile([128, 3, _QSHARD], i8)
                nc.vector.tensor_scalar(
                    ed[:], ps[:, 2048:3584], -_C2, _C2,
                    mybir.AluOpType.max, mybir.AluOpType.add)
                if _ALL_ACT:
                    ea2 = eapool.tile([128, 3, _QSHARD], f8)
                    nc.scalar.activation(ea2[:], ps[:, 2048:3584],
                                         mybir.ActivationFunctionType.Exp,
                                         scale=1.0 / _C1)
                    reduce_dr(ea[:, 0:2, :])
                    reduce_dr(ea[:, 2:4, :])
                    reduce_dr(ea2[:, 0:2, :])
                    reduce_plain(ea2[:, 2, :],
                                 last=(r == nr - 1 and nleft == 0))
                elif _USE_DR:
                    reduce_dr(ea[:, 0:2, :])
                    reduce_dr(ea[:, 2:4, :])
                    reduce_dr(ed[:, 0:2, :].bitcast(f8))
                    reduce_plain(ed[:, 2, :].bitcast(f8),
                                 last=(r == nr - 1 and nleft == 0))
                else:
                    for p in range(4):
                        reduce_plain(ea[:, p, :])
                    for p in range(2):
                        reduce_plain(ed[:, p, :].bitcast(f8))
                    reduce_plain(ed[:, 2, :].bitcast(f8),
                                 last=(r == nr - 1 and nleft == 0))

            if nleft:
                t0 = nr * _RT
                ych = ypool.tile([128, nleft * 128], bf16)
                nc.sync.dma_start(ych[0:_K, :], y_d[:, t0 * 128:])
                for i in range(nleft):
                    nc.tensor.matmul(
                        out=ps[:, i * 512:(i + 1) * 512],
                        lhsT=ych[0:_K, i * 128:(i + 1) * 128],
                        rhs=xsb[0:_K, :],
                        start=True, stop=True,
                        tile_position=(0, 0),
                    )
                ea = eapool.tile([128, nleft, _QSHARD], f8)
                nc.scalar.activation(ea[:], ps[:, 0:nleft * 512],
                                     mybir.ActivationFunctionType.Exp,
                                     scale=1.0 / _C1)
                for i in range(nleft):
                    reduce_plain(ea[:, i, :], last=(i == nleft - 1))

            lg = spool.tile([1, _QSHARD], f32)
            nc.scalar.activation(lg[:], acc[0:1, :],
                                 mybir.ActivationFunctionType.Ln)
            fin = spool.tile([1, _QSHARD], f32)
            nc.vector.tensor_tensor(fin[:], lg[:], dv_sb[:],
                                    mybir.AluOpType.subtract)
            nc.sync.dma_start(out_d[:], fin[:])

    nc.compile()
    return nc


def _get_program(n_trains: int):
    if n_trains not in _prog_cache:
        _prog_cache[n_trains] = _build_program(n_trains)
    return _prog_cache[n_trains]


def _prep_inputs(X, X_train, sample_weight):
    X = np.ascontiguousarray(np.asarray(X, dtype=np.float32))
    Y = np.ascontiguousarray(np.asarray(X_train, dtype=np.float32))
    w = np.ascontiguousarray(np.asarray(sample_weight, dtype=np.float32))
    n = Y.shape[0]

    w64 = w.astype(np.float64)
    b64 = np.log(np.maximum(w64, 1e-300)) - 0.5 * np.sum(
        Y.astype(np.float64) ** 2, axis=1)
    b64 = np.clip(b64, -35.0, None)

    # bias rows carry (C1/4)*b split hi/lo; the x side multiplies them by 4.0
    cb64 = (_C1 / 4.0) * b64
    bhi = cb64.astype(np.float32).astype(_BF16)
    blo = (cb64 - bhi.astype(np.float64)).astype(np.float32).astype(_BF16)
    yext = np.empty((_K, n), dtype=_BF16)
    yext[0:32] = Y.astype(_BF16).T
    yext[32] = bhi
    yext[33] = blo
    yext[34] = np.ones(n, dtype=_BF16)

    const = 0.5 * _D * np.log(2.0 * np.pi) + np.log(np.sum(w64))
    xsq = np.sum(X.astype(np.float64) ** 2, axis=1)
    r = np.sqrt(xsq)
    m_est = _A_FIT * r + _C_FIT
    # shift row in C1-scaled units (psum holds C1*arg); bf16-rounded
    mrow_c = (-_C1 * (m_est + _AOFF)).astype(np.float32).astype(_BF16)  # [Q]
    shift_eff = mrow_c.astype(np.float64) / _C1    # exact effective shift
    # out = ln(total) - (shift + ||x||^2/2 + const)
    dv_all = (shift_eff + 0.5 * xsq + const).astype(np.float32)

    in_maps = []
    for c in range(_NCORES):
        sl = slice(c * _QSHARD, (c + 1) * _QSHARD)
        xq = X[sl]
        xext = np.empty((_K, _QSHARD), dtype=_BF16)
        xext[0:32] = (_C1 * xq.astype(np.float64)).astype(_BF16).T
        xext[32] = np.full(_QSHARD, 4.0, dtype=_BF16)
        xext[33] = np.full(_QSHARD, 4.0, dtype=_BF16)
        xext[34] = mrow_c[sl]
        dv = np.ascontiguousarray(dv_all[sl].reshape(1, _QSHARD))
        ones8 = np.full((128, 256), 0x3c, dtype=np.int8)  # e5m2 1.0
        in_maps.append({"yext": yext, "xext": xext, "dv": dv, "ones8": ones8})
    return in_maps


def _gather(results):
    out = np.empty(_Q, dtype=np.float32)
    for c in range(_NCORES):
        out[c * _QSHARD:(c + 1) * _QSHARD] = results[c]["out"].reshape(-1)
    return out


def kernel(X, X_train, sample_weight, _want_timing=False):
    from concourse.bass_utils import run_bass_kernel_spmd

    nc = _get_program(_N)
    in_maps = _prep_inputs(X, X_train, sample_weight)
    kres = run_bass_kernel_spmd(
        nc, in_maps, core_ids=list(range(_NCORES)),
        trace=bool(_want_timing),
    )
    out = _gather(kres.results)
    if _want_timing:
        return out, kres
    return out


# revision 12
# speedup vs baseline: 1.4264x; 1.1545x over previous
"""Gaussian KDE (brute-force, bandwidth^2 = 1) on 8 Trainium2 NeuronCores.

Math:
    out_i = log( sum_j w_j * exp(-||x_i - y_j||^2 / 2) ) - (d/2) log(2pi) - log(sum_j w_j)
          = log( sum_j exp(x_i . y_j + b_j - m_i) ) + m_i - ||x_i||^2/2 - consts
    with b_j = log(w_j) - ||y_j||^2/2 and m_i a per-query shift that keeps the
    exponentials inside fp8-e5m2 dynamic range.

Flipped-orientation design (trains on PSUM partitions, queries on free dim),
queries sharded 8 ways (512/core), 512 train tiles of 128 per core:
    - scores: per 128-train tile one K=35 matmul; all operands pre-scaled by
      C1 = 4/ln2 so PSUM holds C1*(exp argument) (rows = 32 C1-scaled x dims,
      two 4.0 rows multiplying (C1/4)*b hi/lo, and the -C1*(m_i+AOFF) row).
      K=35 <= 64, so pairs of matmuls run concurrently on disjoint PE row
      groups via tile_position (0,0)/(64,0).
    - exp, supertile = 2 tiles (2 PSUM banks): ScalarE table-exp with
      scale=1/C1 (free) or VectorE tensor_scalar max(-C2)+C2 whose int8
      result IS the e5m2 bit pattern of exp (the clamp makes byte wrap
      impossible); both write fp8-e5m2 tiles to SBUF. Pattern 7:6 balances
      the two engines.
    - reduce over trains: one fp8 DoubleRow ones-matmul per supertile
      (2 train tiles, 0.5 cycles/column) accumulating into a persistent
      PSUM bank; emitted 2 supertiles late so TensorE never waits on exp.
    - final: ln on ScalarE, subtract per-query constant, DMA out.
"""

import numpy as np
import ml_dtypes

_Q, _N, _D = 4096, 65536, 32
_NCORES = 8
_QSHARD = _Q // _NCORES          # 512 queries per core
_K = 35                          # 32 dims + bias hi + bias lo + shift row
_TT = _N // 128                  # 512 train tiles per core
_NSUP = _TT // 2                 # 256 supertiles of 2 tiles

_BF16 = ml_dtypes.bfloat16

# Per-query shift estimate m_i ~= A_FIT*||x_i|| + C_FIT (keeps exp args in
# e5m2 range; exactness is irrelevant - the shift is added back exactly).
_A_FIT = 4.42465707
_C_FIT = -17.07362259
# exp arg = s - m_i - AOFF; AOFF places the largest arg ~1 nat under ln(57344).
_AOFF = 8.0 + 1.0 - float(np.log(57344.0))

# Schraudolph fast-exp producing e5m2 bits: e5m2(e^s) ~ uint8(C1*s + C2).
_C1 = 4.0 / float(np.log(2.0))


def _c2_mean_zero():
    f = (np.arange(100000, dtype=np.float64) + 0.5) / 100000.0
    m0 = np.mean((1.0 + f) * 2.0 ** (-f))
    m1 = np.mean(2.0 ** (-f))
    delta = (m0 - 1.0) / m1
    return float(15 * 4.0 - delta * 4.0)


_C2 = _c2_mean_zero()

# supertile s handled by ACT if _PAT[s % 13] else DVE: 7:6 balances engines
_PAT = tuple(s % 2 == 0 for s in range(13))

_prog_cache: dict = {}


def _build_program(n_trains: int):
    import concourse.bass as bass
    import concourse.tile as tile
    from concourse import bacc, mybir

    f32 = mybir.dt.float32
    bf16 = mybir.dt.bfloat16
    i8 = mybir.dt.int8
    f8 = mybir.dt.float8e5
    DR = mybir.MatmulPerfMode.DoubleRow
    nsup = n_trains // 256

    nc = bacc.Bacc("TRN2", target_bir_lowering=False, debug=False,
                   num_devices=_NCORES)

    y_d = nc.dram_tensor("yext", [_K, n_trains], bf16, kind="ExternalInput")
    on_d = nc.dram_tensor("ones8", [128, 256], i8, kind="ExternalInput")
    x_d = nc.dram_tensor("xext", [_K, _QSHARD], bf16, kind="ExternalInput")
    dv_d = nc.dram_tensor("dv", [1, _QSHARD], f32, kind="ExternalInput")
    out_d = nc.dram_tensor("out", [1, _QSHARD], f32, kind="ExternalOutput")

    with tile.TileContext(nc) as tc:
        with (
            tc.tile_pool(name="const", bufs=1) as cpool,
            tc.tile_pool(name="y", bufs=4) as ypool,
            tc.tile_pool(name="exp", bufs=6) as epool,
            tc.tile_pool(name="small", bufs=2) as spool,
            tc.tile_pool(name="ps", bufs=3, space="PSUM") as ppool,
            tc.tile_pool(name="acc", bufs=1, space="PSUM") as apool,
        ):
            xsb = cpool.tile([128, _QSHARD], bf16)
            nc.sync.dma_start(xsb[0:_K, :], x_d[:])
            nc.sync.dma_start(xsb[64:64 + _K, :], x_d[:])
            dv_sb = cpool.tile([1, _QSHARD], f32)
            nc.sync.dma_start(dv_sb[:], dv_d[:])
            ones_dr = cpool.tile([128, 2, 128], f8)
            nc.sync.dma_start(ones_dr[:].bitcast(i8), on_d[:])

            acc = apool.tile([128, _QSHARD], f32)

            pend = {}          # s -> rhs AP for its (lagged) DR matmul
            first_red = [True]

            def emit_reduce(s, last=False):
                rhs = pend.pop(s)
                nc.tensor.matmul(out=acc[:], lhsT=ones_dr[:], rhs=rhs,
                                 start=first_red[0], stop=last,
                                 perf_mode=DR, skip_group_check=True)
                first_red[0] = False

            LAG = 2
            for s in range(nsup):
                # stream 2 tiles (256 trains) of y per supertile
                ych = ypool.tile([128, 256], bf16)
                nc.sync.dma_start(ych[0:_K, :], y_d[:, s * 256:(s + 1) * 256])
                nc.sync.dma_start(ych[64:64 + _K, :],
                                  y_d[:, s * 256:(s + 1) * 256])
                ps = ppool.tile([128, 2, _QSHARD], f32)
                for h in range(2):
                    rg = 64 * h
                    nc.tensor.matmul(
                        out=ps[:, h, :],
                        lhsT=ych[rg:rg + _K, h * 128:(h + 1) * 128],
                        rhs=xsb[rg:rg + _K, :],
                        start=True, stop=True,
                        tile_position=(rg, 0),
                    )
                if _PAT[s % len(_PAT)]:
                    ex = epool.tile([128, 2, _QSHARD], f8)
                    nc.scalar.activation(ex[:], ps[:],
                                         mybir.ActivationFunctionType.Exp,
                                         scale=1.0 / _C1)
                    pend[s] = ex[:]
                else:
                    ex = epool.tile([128, 2, _QSHARD], i8)
                    nc.vector.tensor_scalar(
                        ex[:], ps[:], -_C2, _C2,
                        mybir.AluOpType.max, mybir.AluOpType.add)
                    pend[s] = ex[:].bitcast(f8)
                if s >= LAG:
                    emit_reduce(s - LAG)
            for s in range(nsup - LAG, nsup):
                emit_reduce(s, last=(s == nsup - 1))

            lg = spool.tile([1, _QSHARD], f32)
            nc.scalar.activation(lg[:], acc[0:1, :],
                                 mybir.ActivationFunctionType.Ln)
            fin = spool.tile([1, _QSHARD], f32)
            nc.vector.tensor_tensor(fin[:], lg[:], dv_sb[:],
                                    mybir.AluOpType.subtract)
            nc.sync.dma_start(out_d[:], fin[:])

    nc.compile()
    return nc


def _get_program(n_trains: int):
    if n_trains not in _prog_cache:
        _prog_cache[n_trains] = _build_program(n_trains)
    return _prog_cache[n_trains]


def _prep_inputs(X, X_train, sample_weight):
    X = np.ascontiguousarray(np.asarray(X, dtype=np.float32))
    Y = np.ascontiguousarray(np.asarray(X_train, dtype=np.float32))
    w = np.ascontiguousarray(np.asarray(sample_weight, dtype=np.float32))
    n = Y.shape[0]

    w64 = w.astype(np.float64)
    b64 = np.log(np.maximum(w64, 1e-300)) - 0.5 * np.sum(
        Y.astype(np.float64) ** 2, axis=1)
    b64 = np.clip(b64, -35.0, None)
    # bias rows carry (C1/4)*b split hi/lo; the x side multiplies them by 4.0
    cb64 = (_C1 / 4.0) * b64
    bhi = cb64.astype(np.float32).astype(_BF16)
    blo = (cb64 - bhi.astype(np.float64)).astype(np.float32).astype(_BF16)
    yext = np.empty((_K, n), dtype=_BF16)
    yext[0:32] = Y.astype(_BF16).T
    yext[32] = bhi
    yext[33] = blo
    yext[34] = np.ones(n, dtype=_BF16)

    const = 0.5 * _D * np.log(2.0 * np.pi) + np.log(np.sum(w64))
    xsq = np.sum(X.astype(np.float64) ** 2, axis=1)
    r = np.sqrt(xsq)
    m_est = _A_FIT * r + _C_FIT
    # shift row in C1-scaled units (psum holds C1*arg); bf16-rounded
    mrow_c = (-_C1 * (m_est + _AOFF)).astype(np.float32).astype(_BF16)  # [Q]
    shift_eff = mrow_c.astype(np.float64) / _C1    # exact effective shift
    # out = ln(total) - (shift + ||x||^2/2 + const)
    dv_all = (shift_eff + 0.5 * xsq + const).astype(np.float32)

    in_maps = []
    for c in range(_NCORES):
        sl = slice(c * _QSHARD, (c + 1) * _QSHARD)
        xq = X[sl]
        xext = np.empty((_K, _QSHARD), dtype=_BF16)
        xext[0:32] = (_C1 * xq.astype(np.float64)).astype(_BF16).T
        xext[32] = np.full(_QSHARD, 4.0, dtype=_BF16)
        xext[33] = np.full(_QSHARD, 4.0, dtype=_BF16)
        xext[34] = mrow_c[sl]
        dv = np.ascontiguousarray(dv_all[sl].reshape(1, _QSHARD))
        ones8 = np.full((128, 256), 0x3c, dtype=np.int8)  # e5m2 1.0
        in_maps.append({"yext": yext, "xext": xext, "dv": dv, "ones8": ones8})
    return in_maps


def _gather(results):
    out = np.empty(_Q, dtype=np.float32)
    for c in range(_NCORES):
        out[c * _QSHARD:(c + 1) * _QSHARD] = results[c]["out"].reshape(-1)
    return out


def kernel(X, X_train, sample_weight, _want_timing=False):
    from concourse.bass_utils import run_bass_kernel_spmd

    nc = _get_program(_N)
    in_maps = _prep_inputs(X, X_train, sample_weight)
    kres = run_bass_kernel_spmd(
        nc, in_maps, core_ids=list(range(_NCORES)),
        trace=bool(_want_timing),
    )
    out = _gather(kres.results)
    if _want_timing:
        return out, kres
    return out


# revision 14
# speedup vs baseline: 1.9779x; 1.3866x over previous
"""Gaussian KDE (brute-force, bandwidth^2 = 1) on 8 Trainium2 NeuronCores.

Math:
    out_i = log( sum_j w_j * exp(-||x_i - y_j||^2 / 2) ) - (d/2) log(2pi) - log(sum_j w_j)
          = log( sum_j exp(x_i . y_j + b_j - m_i) ) + m_i - ||x_i||^2/2 - consts
    with b_j = log(w_j) - ||y_j||^2/2 and m_i a per-query shift that keeps the
    exponentials inside fp8-e5m2 dynamic range.

Flipped-orientation design (trains on PSUM partitions, queries on free dim),
queries sharded 8 ways (512/core), 512 train tiles of 128 per core:
    - scores: per 128-train tile one K=35 matmul; all operands pre-scaled by
      C1 = 4/ln2 so PSUM holds C1*(exp argument) (rows = 32 C1-scaled x dims,
      two 4.0 rows multiplying (C1/4)*b hi/lo, and the -C1*(m_i+AOFF) row).
      K=35 <= 64, so pairs of matmuls run concurrently on disjoint PE row
      groups via tile_position (0,0)/(64,0).
    - exp, supertile = 2 tiles (2 PSUM banks): ScalarE table-exp with
      scale=1/C1 (free) or VectorE tensor_scalar max(-C2)+C2 whose int8
      result IS the e5m2 bit pattern of exp (the clamp makes byte wrap
      impossible); both write fp8-e5m2 tiles to SBUF. Pattern 7:6 balances
      the two engines.
    - reduce over trains: one fp8 DoubleRow ones-matmul per supertile
      (2 train tiles, 0.5 cycles/column) accumulating into a persistent
      PSUM bank; emitted 2 supertiles late so TensorE never waits on exp.
    - final: ln on ScalarE, subtract per-query constant, DMA out.
"""

import numpy as np
import ml_dtypes

_Q, _N, _D = 4096, 65536, 32
_NCORES = 8
_QSHARD = _Q // _NCORES          # 512 queries per core
_K = 35                          # 32 dims + bias hi + bias lo + shift row
_TT = _N // 128                  # 512 train tiles per core
_NSUP = _TT // 2                 # 256 supertiles of 2 tiles

_BF16 = ml_dtypes.bfloat16

# Per-query shift estimate m_i ~= A_FIT*||x_i|| + C_FIT (keeps exp args in
# e5m2 range; exactness is irrelevant - the shift is added back exactly).
_A_FIT = 4.42465707
_C_FIT = -17.07362259
# exp arg = s - m_i - AOFF; AOFF places the largest arg ~1 nat under ln(57344).
_AOFF = 8.0 + 1.0 - float(np.log(57344.0))

# Schraudolph fast-exp producing e5m2 bits: e5m2(e^s) ~ uint8(C1*s + C2).
_C1 = 4.0 / float(np.log(2.0))


def _c2_mean_zero():
    f = (np.arange(100000, dtype=np.float64) + 0.5) / 100000.0
    m0 = np.mean((1.0 + f) * 2.0 ** (-f))
    m1 = np.mean(2.0 ** (-f))
    delta = (m0 - 1.0) / m1
    return float(15 * 4.0 - delta * 4.0)


_C2 = _c2_mean_zero()

# supertile s handled by ACT if _PAT[s % 13] else DVE: 7:6 balances engines
_PAT = tuple(s % 2 == 0 for s in range(13))

_prog_cache: dict = {}


def _build_program(n_trains: int):
    import concourse.bass as bass
    import concourse.tile as tile
    from concourse import bacc, mybir

    f32 = mybir.dt.float32
    bf16 = mybir.dt.bfloat16
    i8 = mybir.dt.int8
    f8 = mybir.dt.float8e5
    DR = mybir.MatmulPerfMode.DoubleRow
    nsup = n_trains // 256

    nc = bacc.Bacc("TRN2", target_bir_lowering=False, debug=False,
                   num_devices=_NCORES)

    y_d = nc.dram_tensor("yext", [_K, n_trains], bf16, kind="ExternalInput")
    on_d = nc.dram_tensor("ones8", [128, 256], i8, kind="ExternalInput")
    x_d = nc.dram_tensor("xext", [_K, _QSHARD], bf16, kind="ExternalInput")
    dv_d = nc.dram_tensor("dv", [1, _QSHARD], f32, kind="ExternalInput")
    out_d = nc.dram_tensor("out", [1, _QSHARD], f32, kind="ExternalOutput")

    with tile.TileContext(nc) as tc:
        with (
            tc.tile_pool(name="const", bufs=1) as cpool,
            tc.tile_pool(name="y", bufs=3) as ypool,
            tc.tile_pool(name="exp", bufs=6) as epool,
            tc.tile_pool(name="small", bufs=2) as spool,
            tc.tile_pool(name="ps", bufs=3, space="PSUM") as ppool,
            tc.tile_pool(name="acc", bufs=1, space="PSUM") as apool,
        ):
            xsb = cpool.tile([128, _QSHARD], bf16)
            nc.sync.dma_start(xsb[0:_K, :], x_d[:])
            nc.sync.dma_start(xsb[64:64 + _K, :], x_d[:])
            dv_sb = cpool.tile([1, _QSHARD], f32)
            nc.sync.dma_start(dv_sb[:], dv_d[:])
            ones_dr = cpool.tile([128, 2, 128], f8)
            nc.sync.dma_start(ones_dr[:].bitcast(i8), on_d[:])

            acc = apool.tile([128, _QSHARD], f32)

            pend = {}          # s -> rhs AP for its (lagged) DR matmul
            first_red = [True]

            def emit_reduce(s, last=False):
                rhs = pend.pop(s)
                nc.tensor.matmul(out=acc[:], lhsT=ones_dr[:], rhs=rhs,
                                 start=first_red[0], stop=last,
                                 perf_mode=DR, skip_group_check=True)
                first_red[0] = False

            LAG = 2
            CH = 16                   # supertiles per y DMA chunk
            ych = None
            for s in range(nsup):
                if s % CH == 0:
                    # stream 16 supertiles (4096 trains) of y per chunk
                    c0 = s * 256
                    ych = ypool.tile([128, CH * 256], bf16)
                    nc.sync.dma_start(ych[0:_K, :],
                                      y_d[:, c0:c0 + CH * 256])
                    nc.sync.dma_start(ych[64:64 + _K, :],
                                      y_d[:, c0:c0 + CH * 256])
                off = (s % CH) * 256
                ps = ppool.tile([128, 2, _QSHARD], f32)
                for h in range(2):
                    rg = 64 * h
                    nc.tensor.matmul(
                        out=ps[:, h, :],
                        lhsT=ych[rg:rg + _K, off + h * 128:off + (h + 1) * 128],
                        rhs=xsb[rg:rg + _K, :],
                        start=True, stop=True,
                        tile_position=(rg, 0),
                    )
                if _PAT[s % len(_PAT)]:
                    ex = epool.tile([128, 2, _QSHARD], f8)
                    nc.scalar.activation(ex[:], ps[:],
                                         mybir.ActivationFunctionType.Exp,
                                         scale=1.0 / _C1)
                    pend[s] = ex[:]
                else:
                    ex = epool.tile([128, 2, _QSHARD], i8)
                    nc.vector.tensor_scalar(
                        ex[:], ps[:], -_C2, _C2,
                        mybir.AluOpType.max, mybir.AluOpType.add)
                    pend[s] = ex[:].bitcast(f8)
                if s >= LAG:
                    emit_reduce(s - LAG)
            for s in range(nsup - LAG, nsup):
                emit_reduce(s, last=(s == nsup - 1))

            lg = spool.tile([1, _QSHARD], f32)
            nc.scalar.activation(lg[:], acc[0:1, :],
                                 mybir.ActivationFunctionType.Ln)
            fin = spool.tile([1, _QSHARD], f32)
            nc.vector.tensor_tensor(fin[:], lg[:], dv_sb[:],
                                    mybir.AluOpType.subtract)
            nc.sync.dma_start(out_d[:], fin[:])

    nc.compile()
    return nc


def _get_program(n_trains: int):
    if n_trains not in _prog_cache:
        _prog_cache[n_trains] = _build_program(n_trains)
    return _prog_cache[n_trains]


def _prep_inputs(X, X_train, sample_weight):
    X = np.ascontiguousarray(np.asarray(X, dtype=np.float32))
    Y = np.ascontiguousarray(np.asarray(X_train, dtype=np.float32))
    w = np.ascontiguousarray(np.asarray(sample_weight, dtype=np.float32))
    n = Y.shape[0]

    w64 = w.astype(np.float64)
    b64 = np.log(np.maximum(w64, 1e-300)) - 0.5 * np.sum(
        Y.astype(np.float64) ** 2, axis=1)
    b64 = np.clip(b64, -35.0, None)
    # bias rows carry (C1/4)*b split hi/lo; the x side multiplies them by 4.0
    cb64 = (_C1 / 4.0) * b64
    bhi = cb64.astype(np.float32).astype(_BF16)
    blo = (cb64 - bhi.astype(np.float64)).astype(np.float32).astype(_BF16)
    yext = np.empty((_K, n), dtype=_BF16)
    yext[0:32] = Y.astype(_BF16).T
    yext[32] = bhi
    yext[33] = blo
    yext[34] = np.ones(n, dtype=_BF16)

    const = 0.5 * _D * np.log(2.0 * np.pi) + np.log(np.sum(w64))
    xsq = np.sum(X.astype(np.float64) ** 2, axis=1)
    r = np.sqrt(xsq)
    m_est = _A_FIT * r + _C_FIT
    # shift row in C1-scaled units (psum holds C1*arg); bf16-rounded
    mrow_c = (-_C1 * (m_est + _AOFF)).astype(np.float32).astype(_BF16)  # [Q]
    shift_eff = mrow_c.astype(np.float64) / _C1    # exact effective shift
    # out = ln(total) - (shift + ||x||^2/2 + const)
    dv_all = (shift_eff + 0.5 * xsq + const).astype(np.float32)

    in_maps = []
    for c in range(_NCORES):
        sl = slice(c * _QSHARD, (c + 1) * _QSHARD)
        xq = X[sl]
        xext = np.empty((_K, _QSHARD), dtype=_BF16)
        xext[0:32] = (_C1 * xq.astype(np.float64)).astype(_BF16).T
        xext[32] = np.full(_QSHARD, 4.0, dtype=_BF16)
        xext[33] = np.full(_QSHARD, 4.0, dtype=_BF16)
        xext[34] = mrow_c[sl]
        dv = np.ascontiguousarray(dv_all[sl].reshape(1, _QSHARD))
        ones8 = np.full((128, 256), 0x3c, dtype=np.int8)  # e5m2 1.0
        in_maps.append({"yext": yext, "xext": xext, "dv": dv, "ones8": ones8})
    return in_maps


def _gather(results):
    out = np.empty(_Q, dtype=np.float32)
    for c in range(_NCORES):
        out[c * _QSHARD:(c + 1) * _QSHARD] = results[c]["out"].reshape(-1)
    return out


def kernel(X, X_train, sample_weight, _want_timing=False):
    from concourse.bass_utils import run_bass_kernel_spmd

    nc = _get_program(_N)
    in_maps = _prep_inputs(X, X_train, sample_weight)
    kres = run_bass_kernel_spmd(
        nc, in_maps, core_ids=list(range(_NCORES)),
        trace=bool(_want_timing),
    )
    out = _gather(kres.results)
    if _want_timing:
        return out, kres
    return out


# revision 15
# speedup vs baseline: 2.0561x; 1.0395x over previous
"""Gaussian KDE (brute-force, bandwidth^2 = 1) on 8 Trainium2 NeuronCores.

Math:
    out_i = log( sum_j w_j * exp(-||x_i - y_j||^2 / 2) ) - (d/2) log(2pi) - log(sum_j w_j)
          = log( sum_j exp(x_i . y_j + b_j - m_i) ) + m_i - ||x_i||^2/2 - consts
    with b_j = log(w_j) - ||y_j||^2/2 and m_i a per-query shift that keeps the
    exponentials inside fp8-e5m2 dynamic range.

Flipped-orientation design (trains on PSUM partitions, queries on free dim),
queries sharded 8 ways (512/core), 512 train tiles of 128 per core:
    - scores: per 128-train tile one K=35 matmul; all operands pre-scaled by
      C1 = 4/ln2 so PSUM holds C1*(exp argument) (rows = 32 C1-scaled x dims,
      two 4.0 rows multiplying (C1/4)*b hi/lo, and the -C1*(m_i+AOFF) row).
      K=35 <= 64, so pairs of matmuls run concurrently on disjoint PE row
      groups via tile_position (0,0)/(64,0).
    - exp, supertile = 2 tiles (2 PSUM banks): ScalarE table-exp with
      scale=1/C1 (free) or VectorE tensor_scalar max(-C2)+C2 whose int8
      result IS the e5m2 bit pattern of exp (the clamp makes byte wrap
      impossible); both write fp8-e5m2 tiles to SBUF. Pattern 7:6 balances
      the two engines.
    - reduce over trains: one fp8 DoubleRow ones-matmul per supertile
      (2 train tiles, 0.5 cycles/column) accumulating into a persistent
      PSUM bank; emitted 2 supertiles late so TensorE never waits on exp.
    - final: ln on ScalarE, subtract per-query constant, DMA out.
"""

import numpy as np
import ml_dtypes

_Q, _N, _D = 4096, 65536, 32
_NCORES = 8
_QSHARD = _Q // _NCORES          # 512 queries per core
_K = 35                          # 32 dims + bias hi + bias lo + shift row
_TT = _N // 128                  # 512 train tiles per core
_NSUP = _TT // 2                 # 256 supertiles of 2 tiles

_BF16 = ml_dtypes.bfloat16

# Per-query shift estimate m_i ~= A_FIT*||x_i|| + C_FIT (keeps exp args in
# e5m2 range; exactness is irrelevant - the shift is added back exactly).
_A_FIT = 4.42465707
_C_FIT = -17.07362259
# exp arg = s - m_i - AOFF; AOFF places the largest arg ~1 nat under ln(57344).
_AOFF = 8.0 + 1.0 - float(np.log(57344.0))

# Schraudolph fast-exp producing e5m2 bits: e5m2(e^s) ~ uint8(C1*s + C2).
_C1 = 4.0 / float(np.log(2.0))


def _c2_mean_zero():
    f = (np.arange(100000, dtype=np.float64) + 0.5) / 100000.0
    m0 = np.mean((1.0 + f) * 2.0 ** (-f))
    m1 = np.mean(2.0 ** (-f))
    delta = (m0 - 1.0) / m1
    return float(15 * 4.0 - delta * 4.0)


_C2 = _c2_mean_zero()

# supertile s handled by ACT if _PAT[s % 13] else DVE: 7:6 balances engines
_PAT = tuple(s % 2 == 0 for s in range(13))

_prog_cache: dict = {}


def _build_program(n_trains: int):
    import concourse.bass as bass
    import concourse.tile as tile
    from concourse import bacc, mybir

    f32 = mybir.dt.float32
    bf16 = mybir.dt.bfloat16
    i8 = mybir.dt.int8
    f8 = mybir.dt.float8e5
    DR = mybir.MatmulPerfMode.DoubleRow
    nsup = n_trains // 256

    nc = bacc.Bacc("TRN2", target_bir_lowering=False, debug=False,
                   num_devices=_NCORES)

    y_d = nc.dram_tensor("yext", [_K, n_trains], bf16, kind="ExternalInput")
    on_d = nc.dram_tensor("ones8", [128, 256], i8, kind="ExternalInput")
    x_d = nc.dram_tensor("xext", [_K, _QSHARD], bf16, kind="ExternalInput")
    dv_d = nc.dram_tensor("dv", [1, _QSHARD], f32, kind="ExternalInput")
    out_d = nc.dram_tensor("out", [1, _QSHARD], f32, kind="ExternalOutput")

    with tile.TileContext(nc) as tc:
        with (
            tc.tile_pool(name="const", bufs=1) as cpool,
            tc.tile_pool(name="y", bufs=3) as ypool,
            tc.tile_pool(name="exp", bufs=6) as epool,
            tc.tile_pool(name="small", bufs=2) as spool,
            tc.tile_pool(name="ps", bufs=3, space="PSUM") as ppool,
            tc.tile_pool(name="acc", bufs=1, space="PSUM") as apool,
        ):
            xsb = cpool.tile([128, _QSHARD], bf16)
            nc.sync.dma_start(xsb[0:_K, :], x_d[:])
            nc.sync.dma_start(xsb[64:64 + _K, :], x_d[:])
            dv_sb = cpool.tile([1, _QSHARD], f32)
            nc.sync.dma_start(dv_sb[:], dv_d[:])
            ones_dr = cpool.tile([128, 2, 128], f8)
            nc.sync.dma_start(ones_dr[:].bitcast(i8), on_d[:])

            acc = apool.tile([128, _QSHARD], f32)

            pend = {}          # s -> rhs AP for its (lagged) DR matmul
            first_red = [True]

            def emit_reduce(s, last=False):
                rhs = pend.pop(s)
                nc.tensor.matmul(out=acc[:], lhsT=ones_dr[:], rhs=rhs,
                                 start=first_red[0], stop=last,
                                 perf_mode=DR, skip_group_check=True)
                first_red[0] = False

            LAG = 4
            CH = 16                   # supertiles per y DMA chunk
            ych = None
            for s in range(nsup):
                if s % CH == 0:
                    # stream 16 supertiles (4096 trains) of y per chunk
                    c0 = s * 256
                    ych = ypool.tile([128, CH * 256], bf16)
                    nc.sync.dma_start(ych[0:_K, :],
                                      y_d[:, c0:c0 + CH * 256])
                    nc.sync.dma_start(ych[64:64 + _K, :],
                                      y_d[:, c0:c0 + CH * 256])
                off = (s % CH) * 256
                ps = ppool.tile([128, 2, _QSHARD], f32)
                for h in range(2):
                    rg = 64 * h
                    nc.tensor.matmul(
                        out=ps[:, h, :],
                        lhsT=ych[rg:rg + _K, off + h * 128:off + (h + 1) * 128],
                        rhs=xsb[rg:rg + _K, :],
                        start=True, stop=True,
                        tile_position=(rg, 0),
                    )
                if _PAT[s % len(_PAT)]:
                    ex = epool.tile([128, 2, _QSHARD], f8)
                    nc.scalar.activation(ex[:], ps[:],
                                         mybir.ActivationFunctionType.Exp,
                                         scale=1.0 / _C1)
                    pend[s] = ex[:]
                else:
                    ex = epool.tile([128, 2, _QSHARD], i8)
                    nc.vector.tensor_scalar(
                        ex[:], ps[:], -_C2, _C2,
                        mybir.AluOpType.max, mybir.AluOpType.add)
                    pend[s] = ex[:].bitcast(f8)
                if s >= LAG:
                    emit_reduce(s - LAG)
            for s in range(nsup - LAG, nsup):
                emit_reduce(s, last=(s == nsup - 1))

            lg = spool.tile([1, _QSHARD], f32)
            nc.scalar.activation(lg[:], acc[0:1, :],
                                 mybir.ActivationFunctionType.Ln)
            fin = spool.tile([1, _QSHARD], f32)
            nc.vector.tensor_tensor(fin[:], lg[:], dv_sb[:],
                                    mybir.AluOpType.subtract)
            nc.sync.dma_start(out_d[:], fin[:])

    nc.compile()
    return nc


def _get_program(n_trains: int):
    if n_trains not in _prog_cache:
        _prog_cache[n_trains] = _build_program(n_trains)
    return _prog_cache[n_trains]


def _prep_inputs(X, X_train, sample_weight):
    X = np.ascontiguousarray(np.asarray(X, dtype=np.float32))
    Y = np.ascontiguousarray(np.asarray(X_train, dtype=np.float32))
    w = np.ascontiguousarray(np.asarray(sample_weight, dtype=np.float32))
    n = Y.shape[0]

    w64 = w.astype(np.float64)
    b64 = np.log(np.maximum(w64, 1e-300)) - 0.5 * np.sum(
        Y.astype(np.float64) ** 2, axis=1)
    b64 = np.clip(b64, -35.0, None)
    # bias rows carry (C1/4)*b split hi/lo; the x side multiplies them by 4.0
    cb64 = (_C1 / 4.0) * b64
    bhi = cb64.astype(np.float32).astype(_BF16)
    blo = (cb64 - bhi.astype(np.float64)).astype(np.float32).astype(_BF16)
    yext = np.empty((_K, n), dtype=_BF16)
    yext[0:32] = Y.astype(_BF16).T
    yext[32] = bhi
    yext[33] = blo
    yext[34] = np.ones(n, dtype=_BF16)

    const = 0.5 * _D * np.log(2.0 * np.pi) + np.log(np.sum(w64))
    xsq = np.sum(X.astype(np.float64) ** 2, axis=1)
    r = np.sqrt(xsq)
    m_est = _A_FIT * r + _C_FIT
    # shift row in C1-scaled units (psum holds C1*arg); bf16-rounded
    mrow_c = (-_C1 * (m_est + _AOFF)).astype(np.float32).astype(_BF16)  # [Q]
    shift_eff = mrow_c.astype(np.float64) / _C1    # exact effective shift
    # out = ln(total) - (shift + ||x||^2/2 + const)
    dv_all = (shift_eff + 0.5 * xsq + const).astype(np.float32)

    in_maps = []
    for c in range(_NCORES):
        sl = slice(c * _QSHARD, (c + 1) * _QSHARD)
        xq = X[sl]
        xext = np.empty((_K, _QSHARD), dtype=_BF16)
        xext[0:32] = (_C1 * xq.astype(np.float64)).astype(_BF16).T
        xext[32] = np.full(_QSHARD, 4.0, dtype=_BF16)
        xext[33] = np.full(_QSHARD, 4.0, dtype=_BF16)
        xext[34] = mrow_c[sl]
        dv = np.ascontiguousarray(dv_all[sl].reshape(1, _QSHARD))
        ones8 = np.full((128, 256), 0x3c, dtype=np.int8)  # e5m2 1.0
        in_maps.append({"yext": yext, "xext": xext, "dv": dv, "ones8": ones8})
    return in_maps


def _gather(results):
    out = np.empty(_Q, dtype=np.float32)
    for c in range(_NCORES):
        out[c * _QSHARD:(c + 1) * _QSHARD] = results[c]["out"].reshape(-1)
    return out


def kernel(X, X_train, sample_weight, _want_timing=False):
    from concourse.bass_utils import run_bass_kernel_spmd

    nc = _get_program(_N)
    in_maps = _prep_inputs(X, X_train, sample_weight)
    kres = run_bass_kernel_spmd(
        nc, in_maps, core_ids=list(range(_NCORES)),
        trace=bool(_want_timing),
    )
    out = _gather(kres.results)
    if _want_timing:
        return out, kres
    return out


# revision 18
# speedup vs baseline: 2.5985x; 1.2638x over previous
"""Gaussian KDE (brute-force, bandwidth^2 = 1) on 8 Trainium2 NeuronCores.

Math:
    out_i = log( sum_j w_j * exp(-||x_i - y_j||^2/2) ) - (d/2) log(2pi) - log(sum_j w_j)
          = log( sum_j exp(x_i . y_j + b_j) ) - ||x_i||^2/2 - consts
    with b_j = log(w_j) - ||y_j||^2/2.

Queries sharded 8 ways (512/core, 4 PSUM-partition tiles). Per core:
    - scores: K=35 bf16 matmuls, stationary = query tile [35, 128], moving =
      train slices. Operands are pre-scaled so PSUM holds C1*s + C2b/32
      (C1 = 4/ln2, C2b the bf16 Schraudolph bias): x rows = C1*x dims plus
      three 4.0 rows; y rows = y dims + (C1/4)*b hi + lo + C2b/128 row.
      K=35 <= 64, so consecutive matmuls alternate PE row groups via
      tile_position (0,0)/(64,0) and run pairwise-concurrently.
    - exp+sum, two unit types balanced by a credit scheduler across engines:
      * ACT unit [128, 1536] (3 banks): table-exp in place with
        scale=1/C1, bias=-C2b/(32*C1), free-dim sum fused via accum_out.
      * DVE unit [128, 512] (1 bank): tensor_scalar (mult 32, max 0) whose
        int16 result IS the bf16 bit pattern of exp (Schraudolph; the max
        makes int16 wrap impossible), then tensor_reduce of the bitcast.
    - final: per query tile reduce partials, ln, subtract per-query const.
"""

import numpy as np
import ml_dtypes

_Q, _N, _D = 4096, 65536, 32
_NCORES = 8
_QSHARD = _Q // _NCORES          # 512 queries per core
_K = 34                          # 32 dims + bias hi/lo (incl C2 const)
_QT = 4                          # query tiles per core

_BF16 = ml_dtypes.bfloat16

_C1 = 4.0 / float(np.log(2.0))


def _c2b():
    f = (np.arange(100000, dtype=np.float64) + 0.5) / 100000.0
    m0 = np.mean((1.0 + f) * 2.0 ** (-f))
    m1 = np.mean(2.0 ** (-f))
    delta = (m0 - 1.0) / m1
    return float(127 * 128 - delta * 128)


_C2B = _c2b()

# per query tile: trains covered by ACT units (1536 each) and DVE units (512)
_NA = 29                         # 29 * 1536 = 44544
_ND = 41                         # 41 * 512  = 20992 ; total 65536
_ACT_NS = 1724.0                 # (1536+192)/1.2 + 284
_DVE_NS = 1252.0                 # (120+512)/0.96 + (58+512)/0.96

_prog_cache: dict = {}


def _unit_schedule():
    """Credit-scheduled unit type sequence (shared by all 4 query tiles)."""
    seq = []
    na, nd, ta, td = 0, 0, 0.0, 0.0
    while na < _NA or nd < _ND:
        if nd >= _ND or (na < _NA and ta <= td):
            seq.append('A')
            na += 1
            ta += _ACT_NS
        else:
            seq.append('D')
            nd += 1
            td += _DVE_NS
    return seq


def _build_program(n_trains: int):
    import concourse.bass as bass
    import concourse.tile as tile
    from concourse import bacc, mybir

    f32 = mybir.dt.float32
    bf16 = mybir.dt.bfloat16
    i16 = mybir.dt.int16

    nc = bacc.Bacc("TRN2", target_bir_lowering=False, debug=False,
                   num_devices=_NCORES)

    y_d = nc.dram_tensor("yext", [_K, n_trains], bf16, kind="ExternalInput")
    x_d = nc.dram_tensor("xext", [_K, _QSHARD], bf16, kind="ExternalInput")
    dv_d = nc.dram_tensor("dv", [128, _QT], f32, kind="ExternalInput")
    out_d = nc.dram_tensor("out", [128, _QT], f32, kind="ExternalOutput")

    seq = _unit_schedule()

    with tile.TileContext(nc) as tc:
        with (
            tc.tile_pool(name="const", bufs=1) as cpool,
            tc.tile_pool(name="q16", bufs=6) as qpool,
            tc.tile_pool(name="small", bufs=2) as spool,
            tc.tile_pool(name="ps", bufs=1, space="PSUM") as ppool,
        ):
            xsb = cpool.tile([128, _QSHARD], bf16)
            nc.sync.dma_start(xsb[0:_K, :], x_d[:])
            nc.sync.dma_start(xsb[64:64 + _K, :], x_d[:])
            dv_sb = cpool.tile([128, _QT], f32)
            nc.sync.dma_start(dv_sb[:], dv_d[:])
            bias_sb = cpool.tile([128, 1], f32)
            nc.vector.memset(bias_sb[:], -_C2B / (32.0 * _C1))

            # y resident in SBUF, both row-group strips, 8 DMA pieces each
            ysb = cpool.tile([128, n_trains], bf16)
            npc = n_trains // 8
            for p in range(8):
                nc.sync.dma_start(ysb[0:_K, p * npc:(p + 1) * npc],
                                  y_d[:, p * npc:(p + 1) * npc])
                nc.sync.dma_start(ysb[64:64 + _K, p * npc:(p + 1) * npc],
                                  y_d[:, p * npc:(p + 1) * npc])

            sall = cpool.tile([128, len(seq) * _QT], f32)

            # PSUM: A units double-buffered at [0:1536],[1536:3072];
            # D units at [3072:3584],[3584:4096]
            ps = ppool.tile([128, 8 * 512], f32)

            rg_par = [0]

            def score_mm(qt, dst, t0, width):
                for j in range(width // 512):
                    rg = 64 * (rg_par[0] & 1)
                    rg_par[0] += 1
                    nc.tensor.matmul(
                        out=ps[:, dst + j * 512: dst + (j + 1) * 512],
                        lhsT=xsb[rg:rg + _K, qt * 128:(qt + 1) * 128],
                        rhs=ysb[rg:rg + _K, t0 + j * 512: t0 + (j + 1) * 512],
                        start=True, stop=True,
                        tile_position=(rg, 0),
                    )

            col = [0]
            gen = {'A': 0, 'D': 0}
            cur = [0] * _QT
            for typ in seq:
                for qt in range(_QT):
                    t0 = cur[qt]
                    if typ == 'A':
                        dst = 1536 * (gen['A'] & 1)
                        gen['A'] += 1
                        score_mm(qt, dst, t0, 1536)
                        nc.scalar.activation(
                            ps[:, dst:dst + 1536], ps[:, dst:dst + 1536],
                            mybir.ActivationFunctionType.Exp,
                            bias=bias_sb[:], scale=1.0 / _C1,
                            accum_out=sall[:, col[0]:col[0] + 1])
                        cur[qt] = t0 + 1536
                    else:
                        dst = 3072 + 512 * (gen['D'] & 1)
                        gen['D'] += 1
                        score_mm(qt, dst, t0, 512)
                        q16 = qpool.tile([128, 512], i16)
                        nc.vector.tensor_scalar(
                            q16[:], ps[:, dst:dst + 512], 32.0, 0.0,
                            mybir.AluOpType.mult, mybir.AluOpType.max)
                        nc.vector.tensor_reduce(
                            sall[:, col[0]:col[0] + 1], q16[:].bitcast(bf16),
                            axis=mybir.AxisListType.X, op=mybir.AluOpType.add)
                        cur[qt] = t0 + 512
                    col[0] += 1

            nun = len(seq)
            fin = spool.tile([128, _QT], f32)
            for qt in range(_QT):
                red = spool.tile([128, 1], f32)
                nc.vector.tensor_reduce(
                    red[:], sall[:, qt:qt + 4 * (nun - 1) + 1:4],
                    axis=mybir.AxisListType.X, op=mybir.AluOpType.add)
                lg = spool.tile([128, 1], f32)
                nc.scalar.activation(lg[:], red[:],
                                     mybir.ActivationFunctionType.Ln)
                nc.vector.tensor_sub(fin[:, qt:qt + 1], lg[:],
                                     dv_sb[:, qt:qt + 1])
            nc.sync.dma_start(out_d[:], fin[:])

    nc.compile()
    return nc


def _get_program(n_trains: int):
    if n_trains not in _prog_cache:
        _prog_cache[n_trains] = _build_program(n_trains)
    return _prog_cache[n_trains]


def _prep_inputs(X, X_train, sample_weight):
    X = np.ascontiguousarray(np.asarray(X, dtype=np.float32))
    Y = np.ascontiguousarray(np.asarray(X_train, dtype=np.float32))
    w = np.ascontiguousarray(np.asarray(sample_weight, dtype=np.float32))
    n = Y.shape[0]

    w64 = w.astype(np.float64)
    b64 = np.log(np.maximum(w64, 1e-300)) - 0.5 * np.sum(
        Y.astype(np.float64) ** 2, axis=1)
    b64 = np.clip(b64, -35.0, None)
    cb64 = (_C1 * b64 + _C2B / 32.0) / 4.0
    bhi = cb64.astype(np.float32).astype(_BF16)
    blo = (cb64 - bhi.astype(np.float64)).astype(np.float32).astype(_BF16)

    yext = np.empty((_K, n), dtype=_BF16)
    yext[0:32] = Y.astype(_BF16).T
    yext[32] = bhi
    yext[33] = blo

    const = 0.5 * _D * np.log(2.0 * np.pi) + np.log(np.sum(w64))
    xsq = np.sum(X.astype(np.float64) ** 2, axis=1)
    dv_all = (0.5 * xsq + const).astype(np.float32)

    in_maps = []
    for c in range(_NCORES):
        sl = slice(c * _QSHARD, (c + 1) * _QSHARD)
        xq = X[sl]
        xext = np.empty((_K, _QSHARD), dtype=_BF16)
        xext[0:32] = (_C1 * xq.astype(np.float64)).astype(_BF16).T
        xext[32] = np.full(_QSHARD, 4.0, dtype=_BF16)
        xext[33] = np.full(_QSHARD, 4.0, dtype=_BF16)
        dv = np.ascontiguousarray(dv_all[sl].reshape(_QT, 128).T)
        in_maps.append({"yext": yext, "xext": xext, "dv": dv})
    return in_maps


def _gather(results):
    out = np.empty(_Q, dtype=np.float32)
    for c in range(_NCORES):
        res = results[c]["out"]                        # [128, QT]
        out[c * _QSHARD:(c + 1) * _QSHARD] = res.T.reshape(_QSHARD)
    return out


def kernel(X, X_train, sample_weight, _want_timing=False):
    from concourse.bass_utils import run_bass_kernel_spmd

    nc = _get_program(_N)
    in_maps = _prep_inputs(X, X_train, sample_weight)
    kres = run_bass_kernel_spmd(
        nc, in_maps, core_ids=list(range(_NCORES)),
        trace=bool(_want_timing),
    )
    out = _gather(kres.results)
    if _want_timing:
        return out, kres
    return out
